# revision 9
# baseline (speedup 1.0000x reference)
# Cross-attention kernel for Trainium2 (Bass/Tile), 8-core data-parallel.
#
# Reference computation (per batch element, B=8 -> one batch element per core):
#   q = x1 @ Wq.T + bq ; k = x2 @ Wk.T + bk ; v = x3 @ Wv.T + bv
#   out = softmax(q @ k.T) @ v          (no 1/sqrt(d) scale)
#
# Precision strategy (validated numerically and on hardware, absmax rel err
# ~1.3e-2 vs the 2e-2 gate):
#   - q,k projections and q@k.T run as SINGLE-pass fp32r matmuls. fp32r is
#     fp32 rounded to 11 explicit mantissa bits; the PE runs it at full bf16
#     rate when the moving free dim is >= 256 and the matmul is exact given
#     the rounded inputs. fp32r operands are produced on DVE/GpSimd (any op
#     with an fp32r output dtype rounds); fp32r == bf16(hi) + bf16(lo)
#     exactly, so transposed fp32r tiles are built from two 2-byte xbar
#     transposes + one recombining add.
#   - v projection and attn@v run in fp16 (~3e-4 contribution).
#   - softmax is fp32 (row max on DVE, exp on ScalarE with accumulate,
#     normalization deferred to the output).
#
# Schedule strategy (the PE is the roofline: 7 passes of S*C*C MACs = 382us
# warm; everything else must hide under it):
#   - Phase order V -> Q -> K -> attention. v and qT spill to DRAM scratch;
#     kT (the score moving operand, fp32r, 64 KiB/partition) is built
#     directly resident in the second scope and v streams back during the K
#     phase. This keeps every phase's SBUF peak under the ~208 KiB budget
#     with strictly LIFO pool lifetimes.
#   - One-chunk lookahead that crosses phase boundaries, shared staging
#     pools, W preps emitted early so they run on DMA/GpSimd/ACT while the
#     PE chews the previous phase's matmuls. Hi/lo split + recombine math
#     runs on GpSimd so the DVE stream (PSUM-draining bias adds) never
#     stalls the PE.
#   - Attention is software-pipelined: scores S(sq) run on the PE while
#     softmax(sq-1) finishes on DVE/ACT, and attn@v A(sq-1) is emitted after
#     S(sq) so the PE never waits for a softmax. Scores run c-outer so PSUM
#     score banks free progressively for the next tile.

from contextlib import ExitStack

import numpy as np

import concourse.bass as bass
import concourse.mybir as mybir
import concourse.tile as tile
from concourse import bacc
from concourse.bass_utils import run_bass_kernel_spmd

F32 = mybir.dt.float32
F32R = mybir.dt.float32r
BF16 = mybir.dt.bfloat16
F16 = mybir.dt.float16
ADD = mybir.AluOpType.add
SUB = mybir.AluOpType.subtract
AX = mybir.AxisListType.X
EXP = mybir.ActivationFunctionType.Exp

B, S, C = 8, 2048, 1024
P = 128
NT_S = S // P  # 16 s-tiles
NT_C = C // P  # 8 c/d-tiles
CH = 512  # free-dim chunk (one fp32 PSUM bank; fp32r full rate needs >=256)
NCH_S = S // CH  # 4
NCH_C = C // CH  # 2
NJ = CH // P  # 4 s-tiles per chunk


def _emit(tc):
    nc = tc.nc

    x1 = nc.dram_tensor("x1", [S, C], F32, kind="ExternalInput").ap()
    x2 = nc.dram_tensor("x2", [S, C], F32, kind="ExternalInput").ap()
    x3 = nc.dram_tensor("x3", [S, C], F32, kind="ExternalInput").ap()
    Wq = nc.dram_tensor("Wq", [C, C], F32, kind="ExternalInput").ap()
    Wk = nc.dram_tensor("Wk", [C, C], F32, kind="ExternalInput").ap()
    Wv = nc.dram_tensor("Wv", [C, C], F32, kind="ExternalInput").ap()
    bq = nc.dram_tensor("bq", [C], F32, kind="ExternalInput").ap()
    bk = nc.dram_tensor("bk", [C], F32, kind="ExternalInput").ap()
    bv = nc.dram_tensor("bv", [C], F32, kind="ExternalInput").ap()
    out = nc.dram_tensor("out", [S, C], F32, kind="ExternalOutput").ap()

    es = ExitStack()
    with es:
        const = es.enter_context(tc.tile_pool(name="const", bufs=1))
        dram = es.enter_context(tc.tile_pool(name="dram", bufs=1, space="DRAM"))

        # biases: bq/bk as per-d-tile columns [128, 8]; bv broadcast [128, C]
        bq_sb = const.tile([P, NT_C], F32, tag="bq")
        nc.scalar.dma_start(out=bq_sb, in_=bq.rearrange("(t p) -> p t", p=P))
        bk_sb = const.tile([P, NT_C], F32, tag="bk")
        nc.scalar.dma_start(out=bk_sb, in_=bk.rearrange("(t p) -> p t", p=P))
        bv_sb = const.tile([P, C], F32, tag="bv")
        bv_bcast = bass.AP(tensor=bv.tensor, offset=bv.offset, ap=[[0, P], [1, C]])
        nc.scalar.dma_start(out=bv_sb, in_=bv_bcast)

        # DRAM scratch: spilled qT (fp32r bits) and v (fp16)
        qT_d = dram.tile([NT_C, P, S], F32R, tag="qTd", name="qTd")
        v_d = dram.tile([NT_S, P, C], F16, tag="vd", name="vd")

        with (
            tc.tile_pool(name="xs", bufs=2) as xs,
            tc.tile_pool(name="xt", bufs=2) as xt,
            tc.tile_pool(name="xs2", bufs=2) as xs2,
        ):

            def prep_w_f32r(W, wpool):
                """Per-dt W^T tiles [128c, NT_C(ct), 128(d)] fp32r."""
                tiles = []
                for dt in range(NT_C):
                    wt = wpool.tile([P, NT_C, P], F32R, tag=f"W{dt}", name=f"W{dt}")
                    wnat = xs.tile([P, C], F32, tag="xload", name="wnat")
                    weng = nc.gpsimd if dt % 2 == 0 else nc.scalar
                    weng.dma_start(out=wnat, in_=W[dt * P : (dt + 1) * P, :])
                    wh = xs.tile([P, C], BF16, tag="xh", name="wh")
                    nc.scalar.copy(out=wh, in_=wnat)
                    wl = xs2.tile([P, C], BF16, tag="xl", name="wl")
                    nc.gpsimd.tensor_tensor(out=wl, in0=wnat, in1=wh, op=SUB)
                    whT = xs2.tile([P, NT_C, P], BF16, tag="xhT", name="whT")
                    nc.sync.dma_start(out=whT, in_=wh, transpose=True)
                    wlT = xs2.tile([P, NT_C, P], BF16, tag="xlT", name="wlT")
                    nc.sync.dma_start(out=wlT, in_=wl, transpose=True)
                    nc.gpsimd.tensor_tensor(out=wt, in0=whT, in1=wlT, op=ADD)
                    tiles.append(wt)
                return tiles

            def prep_w_f16(W, wpool):
                """One W^T tile [128c, NT_C(ct), C(d)] fp16."""
                WT = wpool.tile([P, NT_C, C], F16, tag="Wv", name="Wv")
                for dt in range(NT_C):
                    wnat = xs.tile([P, C], F32, tag="xload", name="wvnat")
                    weng = nc.gpsimd if dt % 2 == 0 else nc.scalar
                    weng.dma_start(out=wnat, in_=W[dt * P : (dt + 1) * P, :])
                    wh = xs.tile([P, C], F16, tag="xh", name="wv16")
                    nc.scalar.copy(out=wh, in_=wnat)
                    nc.sync.dma_start(
                        out=WT[:, :, dt * P : (dt + 1) * P], in_=wh, transpose=True
                    )
                return WT

            def prep_xT_f32r(x, s0):
                """x[s0:s0+CH, :] -> [128c, NT_C(ct), CH(s)] fp32r."""
                xT = xt.tile([P, NT_C, CH], F32R, tag="xT", name="xT")
                for j in range(NJ):
                    r0 = s0 + j * P
                    xsl = xs.tile([P, C], F32, tag="xload", name="xload")
                    xeng = nc.gpsimd if j % 2 == 0 else nc.scalar
                    xeng.dma_start(out=xsl, in_=x[r0 : r0 + P, :])
                    xh = xs.tile([P, C], BF16, tag="xh", name="xh")
                    nc.scalar.copy(out=xh, in_=xsl)
                    xl = xs2.tile([P, C], BF16, tag="xl", name="xl")
                    nc.gpsimd.tensor_tensor(out=xl, in0=xsl, in1=xh, op=SUB)
                    xhT = xs2.tile([P, NT_C, P], BF16, tag="xhT", name="xhT")
                    nc.sync.dma_start(out=xhT, in_=xh, transpose=True)
                    xlT = xs2.tile([P, NT_C, P], BF16, tag="xlT", name="xlT")
                    nc.sync.dma_start(out=xlT, in_=xl, transpose=True)
                    nc.gpsimd.tensor_tensor(
                        out=xT[:, :, j * P : (j + 1) * P], in0=xhT, in1=xlT, op=ADD
                    )
                return xT

            def prep_xT_f16(x, s0):
                """x[s0:s0+CH, :] -> [128c, NT_C(ct), CH(s)] fp16."""
                xT = xt.tile([P, NT_C, CH], F16, tag="xT", name="xT16")
                for j in range(NJ):
                    r0 = s0 + j * P
                    xsl = xs.tile([P, C], F32, tag="xload", name="xload3")
                    xeng = nc.gpsimd if j % 2 == 0 else nc.scalar
                    xeng.dma_start(out=xsl, in_=x[r0 : r0 + P, :])
                    xh = xs.tile([P, C], F16, tag="xh", name="xh3")
                    nc.scalar.copy(out=xh, in_=xsl)
                    nc.sync.dma_start(
                        out=xT[:, :, j * P : (j + 1) * P], in_=xh, transpose=True
                    )
                return xT

            # ---- scope 1: V phase then Q phase (both spill to DRAM) --------
            with (
                tc.tile_pool(name="wq", bufs=1) as wq_pool,
                tc.tile_pool(name="mmps", bufs=4, space="PSUM") as mmps,
            ):
                with (
                    tc.tile_pool(name="wv", bufs=1) as wv_pool,
                    tc.tile_pool(name="vt", bufs=3) as vtp,
                ):
                    WvT = prep_w_f16(Wv, wv_pool)
                    xT_cur = prep_xT_f16(x3, 0)
                    WqT = prep_w_f32r(Wq, wq_pool)  # overlaps V matmuls
                    for ich in range(NCH_S):
                        if ich + 1 < NCH_S:
                            nxt = prep_xT_f16(x3, (ich + 1) * CH)
                        else:
                            nxt = prep_xT_f32r(x1, 0)  # cross-phase prefetch
                        for j in range(NJ):
                            st = ich * NJ + j
                            vstage = vtp.tile([P, C], F16, tag="vt", name="vt")
                            for cch in range(NCH_C):
                                ps = mmps.tile([P, CH], F32, tag="pps", name="vps")
                                for ct in range(NT_C):
                                    nc.tensor.matmul(
                                        ps,
                                        xT_cur[:, ct, j * P : (j + 1) * P],
                                        WvT[:, ct, cch * CH : (cch + 1) * CH],
                                        start=(ct == 0),
                                        stop=(ct == NT_C - 1),
                                    )
                                nc.vector.tensor_tensor(
                                    out=vstage[:, cch * CH : (cch + 1) * CH],
                                    in0=ps,
                                    in1=bv_sb[:, cch * CH : (cch + 1) * CH],
                                    op=ADD,
                                )
                            nc.scalar.dma_start(out=v_d[st], in_=vstage)
                        xT_cur = nxt

                with tc.tile_pool(name="qt", bufs=3) as qtp:
                    for ich in range(NCH_S):
                        if ich + 1 < NCH_S:
                            nxt = prep_xT_f32r(x1, (ich + 1) * CH)
                        else:
                            nxt = prep_xT_f32r(x2, 0)  # cross-phase prefetch
                        s0 = ich * CH
                        for dt in range(NT_C):
                            ps = mmps.tile([P, CH], F32, tag="pps", name="qps")
                            for ct in range(NT_C):
                                nc.tensor.matmul(
                                    ps,
                                    WqT[dt][:, ct, :],
                                    xT_cur[:, ct, :],
                                    start=(ct == 0),
                                    stop=(ct == NT_C - 1),
                                )
                            qtile = qtp.tile([P, CH], F32R, tag="qt", name="qt")
                            nc.vector.tensor_scalar_add(
                                out=qtile, in0=ps, scalar1=bq_sb[:, dt : dt + 1]
                            )
                            nc.scalar.dma_start(
                                out=qT_d[dt, :, s0 : s0 + CH], in_=qtile
                            )
                        xT_cur = nxt

            # ---- scope 2: K phase (kT resident, v streams back) ------------
            with (
                tc.tile_pool(name="resk", bufs=1) as res_k,
                tc.tile_pool(name="resv", bufs=1) as res_v,
            ):
                kT = res_k.tile([P, NT_C, S], F32R, tag="kT", name="kT")
                v_r = res_v.tile([P, NT_S, C], F16, tag="v", name="v")
                nc.gpsimd.dma_start(out=v_r, in_=v_d.rearrange("t p c -> p t c"))
                with (
                    tc.tile_pool(name="wk", bufs=1) as wk_pool,
                    tc.tile_pool(name="mmps2", bufs=4, space="PSUM") as mmps,
                ):
                    WkT = prep_w_f32r(Wk, wk_pool)
                    for ich in range(NCH_S):
                        if ich + 1 < NCH_S:
                            nxt = prep_xT_f32r(x2, (ich + 1) * CH)
                        else:
                            nxt = None
                        s0 = ich * CH
                        for dt in range(NT_C):
                            ps = mmps.tile([P, CH], F32, tag="pps", name="kps")
                            for ct in range(NT_C):
                                nc.tensor.matmul(
                                    ps,
                                    WkT[dt][:, ct, :],
                                    xT_cur[:, ct, :],
                                    start=(ct == 0),
                                    stop=(ct == NT_C - 1),
                                )
                            nc.vector.tensor_scalar_add(
                                out=kT[:, dt, s0 : s0 + CH],
                                in0=ps,
                                scalar1=bk_sb[:, dt : dt + 1],
                            )
                        xT_cur = nxt

                # ---- attention (software-pipelined over sq) ----------------
                with (
                    tc.tile_pool(name="qstream", bufs=2) as qstream,
                    tc.tile_pool(name="spsum", bufs=6, space="PSUM") as spsum,
                    tc.tile_pool(name="opsum", bufs=2, space="PSUM") as opsum,
                    tc.tile_pool(name="attn", bufs=2) as attn,
                    tc.tile_pool(name="stats", bufs=4) as stats,
                ):

                    def load_q(sq):
                        t = qstream.tile(
                            [P, NT_C, P], F32R, tag="qs", name=f"qs{sq}"
                        )
                        nc.scalar.dma_start(
                            out=t,
                            in_=qT_d[:, :, sq * P : (sq + 1) * P].rearrange(
                                "t p s -> p t s"
                            ),
                        )
                        return t

                    def emit_attnv(pT, rinv, sq):
                        ps_o = [
                            opsum.tile([P, CH], F32, tag="o", name=f"o{sq}_{c}")
                            for c in range(NCH_C)
                        ]
                        for skt in range(NT_S):
                            for cch in range(NCH_C):
                                nc.tensor.matmul(
                                    ps_o[cch],
                                    pT[:, skt, :],
                                    v_r[:, skt, cch * CH : (cch + 1) * CH],
                                    start=(skt == 0),
                                    stop=(skt == NT_S - 1),
                                )
                        o_sb = attn.tile([P, C], F32, tag="osb", name="osb")
                        for cch in range(NCH_C):
                            nc.vector.tensor_scalar_mul(
                                out=o_sb[:, cch * CH : (cch + 1) * CH],
                                in0=ps_o[cch],
                                scalar1=rinv,
                            )
                        nc.scalar.dma_start(
                            out=out[sq * P : (sq + 1) * P, :], in_=o_sb
                        )

                    q_cur = load_q(0)
                    prev = None
                    for sq in range(NT_S):
                        q_next = load_q(sq + 1) if sq + 1 < NT_S else None

                        # scores, c-outer: banks free progressively for sq+1
                        ps_s = [
                            spsum.tile([P, CH], F32, tag="s", name=f"s{sq}_{c}")
                            for c in range(NCH_S)
                        ]
                        for c in range(NCH_S):
                            for dt in range(NT_C):
                                nc.tensor.matmul(
                                    ps_s[c],
                                    q_cur[:, dt, :],
                                    kT[:, dt, c * CH : (c + 1) * CH],
                                    start=(dt == 0),
                                    stop=(dt == NT_C - 1),
                                )

                        # softmax (fp32, row-wise over the free dim)
                        mx = stats.tile([P, NCH_S], F32, tag="mx", name="mx")
                        for c in range(NCH_S):
                            nc.vector.reduce_max(
                                out=mx[:, c : c + 1], in_=ps_s[c], axis=AX
                            )
                        negmax = stats.tile(
                            [P, 1], F32, tag="negmax", name="negmax"
                        )
                        nc.vector.reduce_max(
                            out=negmax, in_=mx, axis=AX, negate=True
                        )

                        p_sb = attn.tile([P, S], F16, tag="p", name="p")
                        sums = stats.tile([P, NCH_S], F32, tag="sums", name="sums")
                        for c in range(NCH_S):
                            nc.scalar.activation(
                                out=p_sb[:, c * CH : (c + 1) * CH],
                                in_=ps_s[c],
                                func=EXP,
                                bias=negmax,
                                scale=1.0,
                                accum_out=sums[:, c : c + 1],
                            )
                        rs = stats.tile([P, 1], F32, tag="rs", name="rs")
                        nc.vector.reduce_sum(out=rs, in_=sums, axis=AX)
                        rinv = stats.tile([P, 1], F32, tag="rinv", name="rinv")
                        nc.vector.reciprocal(out=rinv, in_=rs)

                        pT = attn.tile([P, NT_S, P], F16, tag="pT", name="pT")
                        nc.sync.dma_start(out=pT, in_=p_sb, transpose=True)

                        if prev is not None:
                            emit_attnv(*prev)
                        prev = (pT, rinv, sq)
                        q_cur = q_next
                    emit_attnv(*prev)


_BUILT = {}


def _build():
    if "nc" not in _BUILT:
        nc = bacc.Bacc(
            "TRN2",
            target_bir_lowering=False,
            debug=False,
            num_devices=B,
        )
        with tile.TileContext(nc) as tc:
            _emit(tc)
        nc.compile()
        _BUILT["nc"] = nc
    return _BUILT["nc"]


def kernel_with_results(trace=False, **inputs):
    nc = _build()
    in_maps = []
    for i in range(B):
        in_maps.append(
            {
                "x1": np.ascontiguousarray(inputs["x1"][i], dtype=np.float32),
                "x2": np.ascontiguousarray(inputs["x2"][i], dtype=np.float32),
                "x3": np.ascontiguousarray(inputs["x3"][i], dtype=np.float32),
                "Wq": np.ascontiguousarray(inputs["Wq"], dtype=np.float32),
                "Wk": np.ascontiguousarray(inputs["Wk"], dtype=np.float32),
                "Wv": np.ascontiguousarray(inputs["Wv"], dtype=np.float32),
                "bq": np.ascontiguousarray(inputs["bq"], dtype=np.float32),
                "bk": np.ascontiguousarray(inputs["bk"], dtype=np.float32),
                "bv": np.ascontiguousarray(inputs["bv"], dtype=np.float32),
            }
        )
    res = run_bass_kernel_spmd(nc, in_maps, core_ids=list(range(B)), trace=trace)
    outs = np.stack([r["out"] for r in res.results], axis=0).astype(np.float32)
    return outs, res


def kernel(**inputs):
    outs, _ = kernel_with_results(trace=False, **inputs)
    return outs
